# revision 3
# baseline (speedup 1.0000x reference)
"""Trainium2 Bass kernel for nn_CenterAlignment (segment_reduce).

Strategy (class-sharded, zero-collective), v3:
- Host routes rows by 32-class sub-group s = label>>5. Sub-groups are
  bin-packed to (core, position) by descending count octiles so the
  SPMD-shared per-position segment size (max over cores) hugs the
  actual histogram; every row of a class lands on exactly ONE core, so
  each core computes its 128 classes' sums completely locally.
- Host lays the routed rows out in chunk-contiguous SBUF-native order
  and rounds fp32 -> fp8 e4m3 (the loss is insensitive to sum
  precision), so the device streams a quarter of the fp32 bytes.
- Device per core: stream chunks on the two HWDGE queues
  (sync/scalar), plus late chunks on gpsimd (its first transfer pays
  ~8us of Q7 SWDGE boot, so it gets nothing the tensor engine needs
  early). Labels go FIRST on scalar; iota is generated on-device.
  The last few chunks use dedicated SBUF tiles so their DMA issue
  never waits on tensor-engine progress (tile-pool reuse ties DMA
  issue to the consumer ~bufs chunks back; a transient tensor
  slowdown otherwise throttles the stream itself).
- Per chunk: 32-wide one-hot M = (lab==iota) on DVE, psum[sg] +=
  M^T @ X with one fp8 DoubleRow matmul per TWO tiles (segment sizes
  are even so pairs never straddle a boundary), per-sub-group psum ->
  SBUF -> DRAM drain as each segment ends (only the last one is on
  the critical tail).
- Host: un-permute the 8 cores' sums -> [1024,256], run the exact
  fp32 epilogue (mean, momentum, L2 renorm, presence mask, loss) in
  numpy. Counts come from np.bincount (exact).
"""

import ml_dtypes
import numpy as np

import concourse.bacc as bacc
import concourse.mybir as mybir
import concourse.tile as tile
from concourse.bass_utils import run_bass_kernel_spmd

# ---------------------------------------------------------------- constants
B, D, C = 524288, 256, 1000
N_CORES = 8
MOMENTUM = 0.9
SUB = 32                 # classes per sub-group
SPC = 4                  # sub-groups per core
CH = 16                  # row-tiles per full stream chunk
GP_FROM = 6              # 3-lane mode: gpsimd gets entries >= this index
N_FRESH = 4              # trailing entries with dedicated (non-pooled) tiles

_CACHED = {}


def _plan_entries(segs, ch):
    """Split T tiles into DMA entries: two 8-tile ramp entries, then
    ch-tile chunks, trailing remainder as its own entry."""
    T = sum(segs)
    entries = []
    t = 0
    for nt in (8, 8):
        if t + nt <= T:
            entries.append((t, nt))
            t += nt
    while t < T:
        nt = min(ch, T - t)
        entries.append((t, nt))
        t += nt
    return entries


def _build_nc(segs, cfg=None):
    cfg = dict(cfg or {})
    ch = cfg.get("ch", CH)
    n_dma = cfg.get("n_dma", 3)
    dbufs = cfg.get("dbufs", 18)
    mbufs = cfg.get("mbufs", 12)
    gp_from = cfg.get("gp_from", GP_FROM)
    n_fresh = cfg.get("n_fresh", N_FRESH)

    T = sum(segs)
    assert all(s % 2 == 0 and s >= 2 for s in segs)
    bounds = np.cumsum([0] + list(segs))
    entries = _plan_entries(segs, ch)
    n_entries = len(entries)

    f32 = mybir.dt.float32
    bf16 = mybir.dt.bfloat16
    xdt = mybir.dt.float8e4

    nc = bacc.Bacc("TRN2", target_bir_lowering=False)

    # chunk-contiguous layout: entry k partition p is DRAM row k*128+p
    xs = nc.dram_tensor(
        "xs", [n_entries * 128, ch * D], xdt, kind="ExternalInput"
    )
    lab = nc.dram_tensor("lab", [128, T], bf16, kind="ExternalInput")
    sums_out = nc.dram_tensor("sums", [128, D], f32, kind="ExternalOutput")

    with tile.TileContext(nc) as tc:
        with (
            tc.tile_pool(name="const", bufs=1) as cpool,
            tc.tile_pool(name="dst", bufs=dbufs) as dpool,
            tc.tile_pool(name="dstz", bufs=max(n_fresh, 1)) as zpool,
            tc.tile_pool(name="m", bufs=mbufs) as mpool,
            tc.tile_pool(name="acc", bufs=1) as apool,
        ):
            lab_t = cpool.tile([128, T], bf16)
            iota_t = cpool.tile([128, ch, SUB], bf16)
            # labels FIRST on the scalar HWDGE queue (they gate every
            # one-hot + matmul; HWDGE first-byte ~0.6us vs ~8us Q7
            # SWDGE boot); iota is generated on-device
            nc.scalar.dma_start(lab_t[:], lab[:])
            nc.gpsimd.iota(
                iota_t[:].rearrange("p c k -> p (c k)"),
                pattern=[[0, ch], [1, SUB]],
                base=0,
                channel_multiplier=0,
                allow_small_or_imprecise_dtypes=True,
            )

            with tc.tile_pool(name="psum", bufs=1, space="PSUM") as ppool:
                # DoubleRow matmuls require dst base partition 0: one
                # [32, D] psum tile per sub-group
                ps_list = [
                    ppool.tile([SUB, D], f32, name=f"ps{i}", tag=f"ps{i}")
                    for i in range(SPC)
                ]
                sums_t = apool.tile([128, D], f32)

                # lane plan: ramp entry 0 on sync, entry 1 on scalar
                # (behind lab), then alternate; gpsimd (SWDGE) joins
                # from gp_from so its Q7 boot stays off the tensor
                # engine's critical path
                lanes2 = [nc.sync, nc.scalar]
                lanes3 = [nc.gpsimd, nc.sync, nc.scalar]
                plan = []
                for i, (t0, nt) in enumerate(entries):
                    last = i == n_entries - 1
                    if last:
                        lane = nc.scalar
                    elif n_dma < 3 or i < gp_from:
                        lane = lanes2[i % 2]
                    else:
                        lane = lanes3[(i - gp_from) % 3]
                    plan.append((t0, nt, lane))

                for i, (t0, nt, lane) in enumerate(plan):
                    if i >= n_entries - n_fresh:
                        dst = zpool.tile([128, ch, D], xdt, tag="dstz")
                    else:
                        dst = dpool.tile([128, ch, D], xdt, tag="dst")
                    lane.dma_start(
                        dst[:, 0:nt, :].rearrange("p c d -> p (c d)"),
                        xs[i * 128:(i + 1) * 128, 0:nt * D],
                    )
                    m_t = mpool.tile([128, ch, SUB], xdt, tag="m")
                    nc.vector.tensor_tensor(
                        out=m_t[:, 0:nt, :],
                        in0=lab_t[:, t0:t0 + nt]
                        .unsqueeze(2).to_broadcast([128, nt, SUB]),
                        in1=iota_t[:, 0:nt, :],
                        op=mybir.AluOpType.is_equal,
                    )
                    for j in range(0, nt, 2):
                        t = t0 + j
                        sg = int(np.searchsorted(bounds, t, side="right")) - 1
                        is_sg_first = t == bounds[sg]
                        is_sg_last = t == bounds[sg + 1] - 2
                        nc.tensor.matmul(
                            ps_list[sg][:], m_t[:, j:j + 2, :],
                            dst[:, j:j + 2, :],
                            start=is_sg_first,
                            stop=is_sg_last,
                            perf_mode=mybir.MatmulPerfMode.DoubleRow,
                            skip_group_check=True,
                        )
                        if is_sg_last:
                            # drain this sub-group while the stream
                            # continues; only sg3's drain is on the tail
                            nc.vector.tensor_copy(
                                sums_t[SUB * sg:SUB * (sg + 1), :],
                                ps_list[sg][:],
                            )
                            nc.sync.dma_start(
                                sums_out[SUB * sg:SUB * (sg + 1), :],
                                sums_t[SUB * sg:SUB * (sg + 1), :],
                            )

    nc.compile()
    return nc


def _pack_positions(scnt):
    """Assign sub-groups to (core, position) by descending-count
    octiles: position p gets the p-th octile of sorted counts, so
    seg[p] = 2*ceil(max/256) over that octile is tight. Returns
    (segs, assign) with assign[core][pos] = sub-group id."""
    order = np.argsort(-scnt)  # descending by count
    segs = []
    assign = [[None] * SPC for _ in range(N_CORES)]
    for p in range(SPC):
        octile = order[p * N_CORES:(p + 1) * N_CORES]
        mx = int(scnt[octile].max())
        segs.append(max(2, 2 * int(np.ceil(mx / 256.0))))
        for c in range(N_CORES):
            assign[c][p] = int(octile[c])
    return tuple(segs), assign


def _route(x, l, segs, assign, ch):
    """Host-side routing: per core, rows of its 4 assigned sub-groups
    in chunk-contiguous partition-major SBUF layout, plus sub-group-
    relative labels."""
    l = np.asarray(l).astype(np.int64).ravel()
    x = np.asarray(x)
    valid = (l >= 0) & (l < C)
    if not valid.all():
        x = x[valid]
        l = l[valid]
    sub = l >> 5
    order = np.argsort(sub, kind="stable")
    scnt = np.bincount(sub, minlength=SPC * N_CORES)
    starts = np.concatenate([[0], np.cumsum(scnt)])

    T = sum(segs)
    bounds = np.cumsum([0] + list(segs))
    entries = _plan_entries(segs, ch)
    n_entries = len(entries)

    xq = x.astype(ml_dtypes.float8_e4m3fn)

    in_maps = []
    for c in range(N_CORES):
        arr = np.zeros((T * 128, D), dtype=xq.dtype)
        lab_c = np.full(T * 128, -1.0, dtype=np.float32)
        for p in range(SPC):
            s = assign[c][p]
            rows = order[starts[s]:starts[s + 1]]
            n = len(rows)
            off = bounds[p] * 128
            arr[off:off + n] = xq[rows]
            lab_c[off:off + n] = (l[rows] - SUB * s).astype(np.float32)
        arr = arr.reshape(T, 128, D)
        xs_c = np.zeros((n_entries * 128, ch * D), dtype=xq.dtype)
        for k, (t0, nt) in enumerate(entries):
            blk = arr[t0:t0 + nt].transpose(1, 0, 2).reshape(128, nt * D)
            xs_c[k * 128:(k + 1) * 128, 0:nt * D] = blk
        lab_c = np.ascontiguousarray(
            lab_c.reshape(T, 128).T).astype(ml_dtypes.bfloat16)
        in_maps.append({"xs": xs_c, "lab": lab_c})
    return in_maps


def _epilogue(sums, l, center_img, center_skt):
    ll = np.asarray(l).astype(np.int64).ravel()
    ll = ll[(ll >= 0) & (ll < C)]
    counts = np.bincount(ll, minlength=C)[:C].astype(np.float32)
    cimg = np.asarray(center_img, dtype=np.float32)
    cskt = np.asarray(center_skt, dtype=np.float32)
    present = counts > 0
    mean = sums[:C] / np.maximum(counts, 1.0)[:, None]
    upd = cimg * MOMENTUM + mean * (1.0 - MOMENTUM)
    upd = upd / np.linalg.norm(upd, axis=1, keepdims=True)
    new_img = np.where(present[:, None], upd, cimg)
    diff = new_img - cskt
    sq = np.sum(diff * diff, axis=1)
    n_present = max(float(present.sum()), 1.0)
    return np.float32(np.where(present, sq, 0.0).sum() / n_present)


def _run(x, l, center_img, center_skt, cfg=None, trace=False):
    cfg = dict(cfg or {})
    ch = cfg.setdefault("ch", CH)

    ll = np.asarray(l).astype(np.int64).ravel()
    ll = ll[(ll >= 0) & (ll < C)]
    scnt = np.bincount(ll >> 5, minlength=SPC * N_CORES)
    segs, assign = _pack_positions(scnt)

    in_maps = _route(x, l, segs, assign, ch)

    key = (segs, ch, cfg.get("n_dma", 3), cfg.get("dbufs", 18),
           cfg.get("mbufs", 12), cfg.get("gp_from", GP_FROM),
           cfg.get("n_fresh", N_FRESH))
    if key not in _CACHED:
        _CACHED[key] = _build_nc(segs, cfg)
    nc = _CACHED[key]

    res = run_bass_kernel_spmd(
        nc, in_maps, core_ids=list(range(N_CORES)), trace=trace
    )
    # un-permute: core c psum block p holds sub-group assign[c][p]
    sums = np.zeros((SPC * N_CORES * SUB, D), np.float32)
    for c in range(N_CORES):
        rc = res.results[c]["sums"].astype(np.float32)
        for p in range(SPC):
            s = assign[c][p]
            sums[SUB * s:SUB * (s + 1)] = rc[SUB * p:SUB * (p + 1)]
    loss = _epilogue(sums, l, center_img, center_skt)
    return loss, res


def kernel(x, l, center_img, center_skt):
    loss, _ = _run(x, l, center_img, center_skt)
    return np.asarray(loss, dtype=np.float32).reshape(())


# revision 14
# speedup vs baseline: 1.0396x; 1.0396x over previous
"""Trainium2 Bass kernel for nn_CenterAlignment (segment_reduce).

Strategy (class-sharded, zero-collective), v3:
- Host routes rows by 32-class sub-group s = label>>5. Sub-groups are
  bin-packed to (core, position) by descending count octiles so the
  SPMD-shared per-position segment size (max over cores) hugs the
  actual histogram; every row of a class lands on exactly ONE core, so
  each core computes its 128 classes' sums completely locally.
- Host lays the routed rows out in chunk-contiguous SBUF-native order
  and rounds fp32 -> fp8 e4m3 (the loss is insensitive to sum
  precision), so the device streams a quarter of the fp32 bytes.
- Device per core: stream chunks on the two HWDGE queues
  (sync/scalar), plus late chunks on gpsimd (its first transfer pays
  ~8us of Q7 SWDGE boot, so it gets nothing the tensor engine needs
  early). Labels go FIRST on scalar; iota is generated on-device.
  The last few chunks use dedicated SBUF tiles so their DMA issue
  never waits on tensor-engine progress (tile-pool reuse ties DMA
  issue to the consumer ~bufs chunks back; a transient tensor
  slowdown otherwise throttles the stream itself).
- Per chunk: 32-wide one-hot M = (lab==iota) on DVE, psum[sg] +=
  M^T @ X with one fp8 DoubleRow matmul per TWO tiles (segment sizes
  are even so pairs never straddle a boundary), per-sub-group psum ->
  SBUF -> DRAM drain as each segment ends (only the last one is on
  the critical tail).
- Host: un-permute the 8 cores' sums -> [1024,256], run the exact
  fp32 epilogue (mean, momentum, L2 renorm, presence mask, loss) in
  numpy. Counts come from np.bincount (exact).
"""

import ml_dtypes
import numpy as np

import concourse.bacc as bacc
import concourse.mybir as mybir
import concourse.tile as tile
from concourse.bass_utils import run_bass_kernel_spmd

# ---------------------------------------------------------------- constants
B, D, C = 524288, 256, 1000
N_CORES = 8
MOMENTUM = 0.9
SUB = 32                 # classes per sub-group
SPC = 4                  # sub-groups per core
CH = 16                  # row-tiles per full stream chunk
GP_FROM = 6              # 3-lane mode: gpsimd gets entries >= this index
N_FRESH = 0              # trailing entries with dedicated (non-pooled) tiles

_CACHED = {}


def _plan_entries(segs, ch):
    """Split T tiles into DMA entries: small ramp entries first (low
    latency to the first matmuls), then ch-tile chunks, trailing
    remainder as its own entry."""
    T = sum(segs)
    entries = []
    t = 0
    for nt in (8, 8, 16, 16):
        if nt >= ch:
            break
        if t + nt <= T:
            entries.append((t, nt))
            t += nt
    while t < T:
        nt = min(ch, T - t)
        entries.append((t, nt))
        t += nt
    return entries


def _build_nc(segs, cfg=None):
    cfg = dict(cfg or {})
    ch = cfg.get("ch", CH)
    n_dma = cfg.get("n_dma", 2)
    dbufs = cfg.get("dbufs", 18)
    mbufs = cfg.get("mbufs", 12)
    gp_from = cfg.get("gp_from", GP_FROM)
    n_fresh = cfg.get("n_fresh", N_FRESH)

    T = sum(segs)
    assert all(s % 2 == 0 and s >= 2 for s in segs)
    bounds = np.cumsum([0] + list(segs))
    entries = _plan_entries(segs, ch)
    n_entries = len(entries)

    f32 = mybir.dt.float32
    bf16 = mybir.dt.bfloat16
    xdt = mybir.dt.float8e4

    nc = bacc.Bacc("TRN2", target_bir_lowering=False)

    # chunk-contiguous layout: entry k partition p is DRAM row k*128+p
    xs = nc.dram_tensor(
        "xs", [n_entries * 128, ch * D], xdt, kind="ExternalInput"
    )
    head = min(32, T)
    u8 = mybir.dt.uint8
    lab_h = nc.dram_tensor("lab_h", [128, head], u8, kind="ExternalInput")
    lab_r = (nc.dram_tensor("lab_r", [128, T - head], u8,
                            kind="ExternalInput") if T > head else None)
    iota = nc.dram_tensor("iota", [128, ch * SUB], u8, kind="ExternalInput")
    sums_out = nc.dram_tensor("sums", [128, D], f32, kind="ExternalOutput")

    with tile.TileContext(nc) as tc:
        with (
            tc.tile_pool(name="const", bufs=1) as cpool,
            tc.tile_pool(name="dst", bufs=dbufs) as dpool,
            tc.tile_pool(name="dstz", bufs=max(n_fresh, 1)) as zpool,
            tc.tile_pool(name="m", bufs=mbufs) as mpool,
            tc.tile_pool(name="acc", bufs=1) as apool,
        ):
            lab_t = cpool.tile([128, T], u8)
            iota_t = cpool.tile([128, ch, SUB], u8)
            if cfg.get("primer", 0):
                # tiny first transfer per HWDGE queue: activates the
                # ring while the real constants' descriptors generate
                prim = cpool.tile([128, 64], xdt)
                nc.sync.dma_start(prim[:, 0:32], xs[0:128, 0:32])
                nc.scalar.dma_start(prim[:, 32:64], xs[0:128, 32:64])
            # constants gate every one-hot + matmul. The label head (a
            # few KB, covers the ramp entries) and iota go FIRST on the
            # sync HWDGE ring (the first ring to start draining); the
            # label tail goes first on scalar. Tile's range-precise
            # dependency tracking lets early one-hots run off the head
            # alone.
            nc.sync.dma_start(lab_t[:, 0:head], lab_h[:])
            nc.sync.dma_start(
                iota_t[:].rearrange("p c k -> p (c k)"), iota[:]
            )
            if lab_r is not None:
                nc.scalar.dma_start(lab_t[:, head:T], lab_r[:])

            with tc.tile_pool(name="psum", bufs=1, space="PSUM") as ppool:
                # DoubleRow matmuls require dst base partition 0: one
                # [32, D] psum tile per sub-group
                ps_list = [
                    ppool.tile([SUB, D], f32, name=f"ps{i}", tag=f"ps{i}")
                    for i in range(SPC)
                ]
                warm = cfg.get("warm", 0)
                if warm:
                    ps_w = ppool.tile([SUB, D], f32, name="psw", tag="psw")
                sums_t = apool.tile([128, D], f32)

                # lane plan: ramp entry 0 on sync, entry 1 on scalar
                # (behind lab), then alternate between the two HWDGE
                # queues. gpsimd (SWDGE) is kept OUT of x streaming:
                # its ~8us Q7 boot makes its first transfer complete
                # late, which holds a DMAHW semaphore lane hostage and
                # stalls every DMA 8 program-positions later.
                lanes2 = [nc.scalar, nc.sync]
                lanes3 = [nc.gpsimd, nc.sync, nc.scalar]
                plan = []
                for i, (t0, nt) in enumerate(entries):
                    last = i == n_entries - 1
                    if last:
                        lane = nc.scalar
                    elif n_dma < 3 or i < gp_from:
                        lane = lanes2[i % 2]
                    else:
                        lane = lanes3[(i - gp_from) % 3]
                    plan.append((t0, nt, lane))

                for i, (t0, nt, lane) in enumerate(plan):
                    if i >= n_entries - n_fresh:
                        dst = zpool.tile([128, ch, D], xdt, tag="dstz")
                    else:
                        dst = dpool.tile([128, ch, D], xdt, tag="dst")
                    lane.dma_start(
                        dst[:, 0:nt, :].rearrange("p c d -> p (c d)"),
                        xs[i * 128:(i + 1) * 128, 0:nt * D],
                    )
                    m_t = mpool.tile([128, ch, SUB], xdt, tag="m")
                    nc.vector.tensor_tensor(
                        out=m_t[:, 0:nt, :],
                        in0=lab_t[:, t0:t0 + nt]
                        .unsqueeze(2).to_broadcast([128, nt, SUB]),
                        in1=iota_t[:, 0:nt, :],
                        op=mybir.AluOpType.is_equal,
                    )
                    for j in range(0, nt, 2):
                        t = t0 + j
                        sg = int(np.searchsorted(bounds, t, side="right")) - 1
                        is_sg_first = t == bounds[sg]
                        is_sg_last = t == bounds[sg + 1] - 2
                        nc.tensor.matmul(
                            ps_list[sg][:], m_t[:, j:j + 2, :],
                            dst[:, j:j + 2, :],
                            start=is_sg_first,
                            stop=is_sg_last,
                            perf_mode=mybir.MatmulPerfMode.DoubleRow,
                            skip_group_check=True,
                        )
                        if warm and j + 2 >= nt:
                            # HAM warm-keeper: one throwaway matmul per
                            # chunk fills the PE idle gap between chunks
                            # so the clock gate stays at 8/8 (idle-gap
                            # oscillation otherwise halves the PE clock
                            # for multi-us stretches)
                            nc.tensor.matmul(
                                ps_w[:], m_t[:, j:j + 2, :],
                                dst[:, j:j + 2, :],
                                start=True, stop=True,
                                perf_mode=mybir.MatmulPerfMode.DoubleRow,
                                skip_group_check=True,
                            )
                        if is_sg_last:
                            # drain this sub-group's psum to SBUF while
                            # the stream continues (the DMA out happens
                            # at the end: a mid-stream dma_start whose
                            # wait is on DVE progress would block the
                            # issuing sequencer and stall every x chunk
                            # queued behind it)
                            nc.vector.tensor_copy(
                                sums_t[SUB * sg:SUB * (sg + 1), :],
                                ps_list[sg][:],
                            )
                if warm:
                    wsink = apool.tile([SUB, 4], f32)
                    nc.vector.tensor_copy(wsink[:], ps_w[:, 0:4])
                # per-sub-group output DMAs, issued after all x chunk
                # DMAs: sg0-2 fire immediately (their copies are long
                # done), only sg3's is on the critical tail
                for sg in range(SPC):
                    nc.sync.dma_start(
                        sums_out[SUB * sg:SUB * (sg + 1), :],
                        sums_t[SUB * sg:SUB * (sg + 1), :],
                    )

    nc.compile()
    return nc


def _pack_positions(scnt):
    """Assign sub-groups to (core, position) by descending-count
    octiles: position p gets the p-th octile of sorted counts, so
    seg[p] = 2*ceil(max/256) over that octile is tight. Returns
    (segs, assign) with assign[core][pos] = sub-group id."""
    order = np.argsort(-scnt)  # descending by count
    segs = []
    assign = [[None] * SPC for _ in range(N_CORES)]
    for p in range(SPC):
        octile = order[p * N_CORES:(p + 1) * N_CORES]
        mx = int(scnt[octile].max())
        segs.append(max(2, 2 * int(np.ceil(mx / 256.0))))
        for c in range(N_CORES):
            assign[c][p] = int(octile[c])
    return tuple(segs), assign


def _route(x, l, segs, assign, ch):
    """Host-side routing: per core, rows of its 4 assigned sub-groups
    in chunk-contiguous partition-major SBUF layout, plus sub-group-
    relative labels."""
    l = np.asarray(l).astype(np.int64).ravel()
    x = np.asarray(x)
    valid = (l >= 0) & (l < C)
    if not valid.all():
        x = x[valid]
        l = l[valid]
    sub = l >> 5
    order = np.argsort(sub, kind="stable")
    scnt = np.bincount(sub, minlength=SPC * N_CORES)
    starts = np.concatenate([[0], np.cumsum(scnt)])

    T = sum(segs)
    bounds = np.cumsum([0] + list(segs))
    entries = _plan_entries(segs, ch)
    n_entries = len(entries)

    xq = x.astype(ml_dtypes.float8_e4m3fn)
    head = min(32, T)
    iota_np = np.ascontiguousarray(
        np.tile(np.arange(SUB, dtype=np.uint8), (128, ch)))

    in_maps = []
    for c in range(N_CORES):
        arr = np.zeros((T * 128, D), dtype=xq.dtype)
        lab_c = np.full(T * 128, 255, dtype=np.int64)
        for p in range(SPC):
            s = assign[c][p]
            rows = order[starts[s]:starts[s + 1]]
            n = len(rows)
            off = bounds[p] * 128
            arr[off:off + n] = xq[rows]
            lab_c[off:off + n] = l[rows] - SUB * s
        arr = arr.reshape(T, 128, D)
        xs_c = np.zeros((n_entries * 128, ch * D), dtype=xq.dtype)
        for k, (t0, nt) in enumerate(entries):
            blk = arr[t0:t0 + nt].transpose(1, 0, 2).reshape(128, nt * D)
            xs_c[k * 128:(k + 1) * 128, 0:nt * D] = blk
        lab_c = np.ascontiguousarray(
            lab_c.reshape(T, 128).T).astype(np.uint8)
        im = {"xs": xs_c,
              "lab_h": np.ascontiguousarray(lab_c[:, 0:head]),
              "iota": iota_np}
        if T > head:
            im["lab_r"] = np.ascontiguousarray(lab_c[:, head:T])
        in_maps.append(im)
    return in_maps


def _epilogue(sums, l, center_img, center_skt):
    ll = np.asarray(l).astype(np.int64).ravel()
    ll = ll[(ll >= 0) & (ll < C)]
    counts = np.bincount(ll, minlength=C)[:C].astype(np.float32)
    cimg = np.asarray(center_img, dtype=np.float32)
    cskt = np.asarray(center_skt, dtype=np.float32)
    present = counts > 0
    mean = sums[:C] / np.maximum(counts, 1.0)[:, None]
    upd = cimg * MOMENTUM + mean * (1.0 - MOMENTUM)
    upd = upd / np.linalg.norm(upd, axis=1, keepdims=True)
    new_img = np.where(present[:, None], upd, cimg)
    diff = new_img - cskt
    sq = np.sum(diff * diff, axis=1)
    n_present = max(float(present.sum()), 1.0)
    return np.float32(np.where(present, sq, 0.0).sum() / n_present)


def _run(x, l, center_img, center_skt, cfg=None, trace=False):
    cfg = dict(cfg or {})
    ch = cfg.setdefault("ch", CH)

    ll = np.asarray(l).astype(np.int64).ravel()
    ll = ll[(ll >= 0) & (ll < C)]
    scnt = np.bincount(ll >> 5, minlength=SPC * N_CORES)
    segs, assign = _pack_positions(scnt)

    in_maps = _route(x, l, segs, assign, ch)

    key = (segs, ch, cfg.get("n_dma", 2), cfg.get("dbufs", 18),
           cfg.get("mbufs", 12), cfg.get("gp_from", GP_FROM),
           cfg.get("n_fresh", N_FRESH), cfg.get("warm", 0),
           cfg.get("primer", 0))
    if key not in _CACHED:
        _CACHED[key] = _build_nc(segs, cfg)
    nc = _CACHED[key]

    res = run_bass_kernel_spmd(
        nc, in_maps, core_ids=list(range(N_CORES)), trace=trace
    )
    # un-permute: core c psum block p holds sub-group assign[c][p]
    sums = np.zeros((SPC * N_CORES * SUB, D), np.float32)
    for c in range(N_CORES):
        rc = res.results[c]["sums"].astype(np.float32)
        for p in range(SPC):
            s = assign[c][p]
            sums[SUB * s:SUB * (s + 1)] = rc[SUB * p:SUB * (p + 1)]
    loss = _epilogue(sums, l, center_img, center_skt)
    return loss, res


def kernel(x, l, center_img, center_skt):
    loss, _ = _run(x, l, center_img, center_skt)
    return np.asarray(loss, dtype=np.float32).reshape(())


# revision 15
# speedup vs baseline: 1.0513x; 1.0113x over previous
"""Trainium2 Bass kernel for nn_CenterAlignment (segment_reduce).

Strategy (class-sharded, zero-collective):
- Host routes rows by 32-class sub-group s = label>>5. Sub-groups are
  bin-packed to (core, position) by descending-count octiles so the
  SPMD-shared per-position segment size (max over cores) hugs the
  actual histogram (~3% padding); every row of a class lands on
  exactly ONE core, so each core computes its 128 classes' sums
  completely locally - no cross-core reduction.
- Host lays the routed rows out in chunk-contiguous SBUF-native order
  (each 16-tile chunk is one contiguous 512 KiB DRAM block) and
  rounds fp32 -> fp8 e4m3 (the loss is insensitive to sum precision,
  rel err ~2e-7), so the device streams a quarter of the fp32 bytes
  at the HBM roofline (~380-390 GB/s/core measured).
- Device per core: stream chunks alternating between the two HWDGE
  queues (sync/scalar). gpsimd/SWDGE is kept out of streaming: its
  ~8us Q7 boot makes its first transfer complete late, holding one of
  the 8 round-robin DMAHW completion-semaphore lanes hostage and
  stalling every DMA 8 program-positions later. The uint8 label head
  (+iota) goes first on sync and the label tail first on scalar, so
  the one-hot pipeline starts as early as the ~11us first-DMA-
  completion floor allows.
- Per chunk: 32-wide one-hot M = (lab==iota) on DVE (uint8 in, fp8
  out), psum[sg] += M^T @ X with one fp8 DoubleRow matmul per TWO
  tiles (segment sizes are even so pairs never straddle a boundary),
  per-sub-group psum -> SBUF copy as each segment ends. All output
  DMAs are issued AFTER the x-chunk DMAs (a mid-stream dma_start
  waiting on DVE progress blocks the issuing sequencer and stalls
  every x chunk queued behind it); only sg3's is on the critical
  tail.
- Host: un-permute the 8 cores' sums -> [1024,256], run the exact
  fp32 epilogue (mean, momentum, L2 renorm, presence mask, loss) in
  numpy. Counts come from np.bincount (exact).

Measured dead ends: 3rd DMA lane via gpsimd (Q7-boot lane-hostage,
+2-9us), mid-stream sums DMAs (sequencer stall, +8us), dedicated
no-pool tiles for late chunks (the dependency-driven scheduler
reorders their DMAs to the FRONT, starving the ramp), ch=32 chunks
(+2us median), HAM warm-keeper dummy matmuls (cold-MM count
unchanged), primer DMAs to pre-activate the HWDGE rings (first-byte
time unchanged at ~8.6us).
"""

import ml_dtypes
import numpy as np

import concourse.bacc as bacc
import concourse.mybir as mybir
import concourse.tile as tile
from concourse.bass_utils import run_bass_kernel_spmd

# ---------------------------------------------------------------- constants
B, D, C = 524288, 256, 1000
N_CORES = 8
MOMENTUM = 0.9
SUB = 32                 # classes per sub-group
SPC = 4                  # sub-groups per core
CH = 16                  # row-tiles per full stream chunk
GP_FROM = 6              # 3-lane mode: gpsimd gets entries >= this index
N_FRESH = 0              # trailing entries with dedicated (non-pooled) tiles

_CACHED = {}


def _plan_entries(segs, ch):
    """Split T tiles into DMA entries: small ramp entries first (low
    latency to the first matmuls), then ch-tile chunks, trailing
    remainder as its own entry."""
    T = sum(segs)
    entries = []
    t = 0
    for nt in (8, 8, 16, 16):
        if nt >= ch:
            break
        if t + nt <= T:
            entries.append((t, nt))
            t += nt
    while t < T:
        nt = min(ch, T - t)
        entries.append((t, nt))
        t += nt
    return entries


def _build_nc(segs, cfg=None):
    cfg = dict(cfg or {})
    ch = cfg.get("ch", CH)
    n_dma = cfg.get("n_dma", 2)
    dbufs = cfg.get("dbufs", 18)
    mbufs = cfg.get("mbufs", 12)
    gp_from = cfg.get("gp_from", GP_FROM)
    n_fresh = cfg.get("n_fresh", N_FRESH)

    T = sum(segs)
    assert all(s % 2 == 0 and s >= 2 for s in segs)
    bounds = np.cumsum([0] + list(segs))
    entries = _plan_entries(segs, ch)
    n_entries = len(entries)

    f32 = mybir.dt.float32
    bf16 = mybir.dt.bfloat16
    xdt = mybir.dt.float8e4

    nc = bacc.Bacc("TRN2", target_bir_lowering=False)

    # chunk-contiguous layout: entry k partition p is DRAM row k*128+p
    xs = nc.dram_tensor(
        "xs", [n_entries * 128, ch * D], xdt, kind="ExternalInput"
    )
    head = min(32, T)
    u8 = mybir.dt.uint8
    lab_h = nc.dram_tensor("lab_h", [128, head], u8, kind="ExternalInput")
    lab_r = (nc.dram_tensor("lab_r", [128, T - head], u8,
                            kind="ExternalInput") if T > head else None)
    iota = nc.dram_tensor("iota", [128, ch * SUB], u8, kind="ExternalInput")
    sums_out = nc.dram_tensor("sums", [128, D], f32, kind="ExternalOutput")

    with tile.TileContext(nc) as tc:
        with (
            tc.tile_pool(name="const", bufs=1) as cpool,
            tc.tile_pool(name="dst", bufs=dbufs) as dpool,
            tc.tile_pool(name="dstz", bufs=max(n_fresh, 1)) as zpool,
            tc.tile_pool(name="m", bufs=mbufs) as mpool,
            tc.tile_pool(name="acc", bufs=1) as apool,
        ):
            lab_t = cpool.tile([128, T], u8)
            iota_t = cpool.tile([128, ch, SUB], u8)
            if cfg.get("primer", 0):
                # tiny first transfer per HWDGE queue: activates the
                # ring while the real constants' descriptors generate
                prim = cpool.tile([128, 64], xdt)
                nc.sync.dma_start(prim[:, 0:32], xs[0:128, 0:32])
                nc.scalar.dma_start(prim[:, 32:64], xs[0:128, 32:64])
            # constants gate every one-hot + matmul. The label head (a
            # few KB, covers the ramp entries) and iota go FIRST on the
            # sync HWDGE ring (the first ring to start draining); the
            # label tail goes first on scalar. Tile's range-precise
            # dependency tracking lets early one-hots run off the head
            # alone.
            nc.sync.dma_start(lab_t[:, 0:head], lab_h[:])
            nc.sync.dma_start(
                iota_t[:].rearrange("p c k -> p (c k)"), iota[:]
            )
            if lab_r is not None:
                nc.scalar.dma_start(lab_t[:, head:T], lab_r[:])

            with tc.tile_pool(name="psum", bufs=1, space="PSUM") as ppool:
                # DoubleRow matmuls require dst base partition 0: one
                # [32, D] psum tile per sub-group
                ps_list = [
                    ppool.tile([SUB, D], f32, name=f"ps{i}", tag=f"ps{i}")
                    for i in range(SPC)
                ]
                warm = cfg.get("warm", 0)
                if warm:
                    ps_w = ppool.tile([SUB, D], f32, name="psw", tag="psw")
                sums_t = apool.tile([128, D], f32)

                # lane plan: ramp entry 0 on sync, entry 1 on scalar
                # (behind lab), then alternate between the two HWDGE
                # queues. gpsimd (SWDGE) is kept OUT of x streaming:
                # its ~8us Q7 boot makes its first transfer complete
                # late, which holds a DMAHW semaphore lane hostage and
                # stalls every DMA 8 program-positions later.
                lanes2 = [nc.scalar, nc.sync]
                lanes3 = [nc.gpsimd, nc.sync, nc.scalar]
                plan = []
                for i, (t0, nt) in enumerate(entries):
                    last = i == n_entries - 1
                    if last:
                        lane = nc.scalar
                    elif n_dma < 3 or i < gp_from:
                        lane = lanes2[i % 2]
                    else:
                        lane = lanes3[(i - gp_from) % 3]
                    plan.append((t0, nt, lane))

                for i, (t0, nt, lane) in enumerate(plan):
                    if i >= n_entries - n_fresh:
                        dst = zpool.tile([128, ch, D], xdt, tag="dstz")
                    else:
                        dst = dpool.tile([128, ch, D], xdt, tag="dst")
                    lane.dma_start(
                        dst[:, 0:nt, :].rearrange("p c d -> p (c d)"),
                        xs[i * 128:(i + 1) * 128, 0:nt * D],
                    )
                    m_t = mpool.tile([128, ch, SUB], xdt, tag="m")
                    nc.vector.tensor_tensor(
                        out=m_t[:, 0:nt, :],
                        in0=lab_t[:, t0:t0 + nt]
                        .unsqueeze(2).to_broadcast([128, nt, SUB]),
                        in1=iota_t[:, 0:nt, :],
                        op=mybir.AluOpType.is_equal,
                    )
                    for j in range(0, nt, 2):
                        t = t0 + j
                        sg = int(np.searchsorted(bounds, t, side="right")) - 1
                        is_sg_first = t == bounds[sg]
                        is_sg_last = t == bounds[sg + 1] - 2
                        nc.tensor.matmul(
                            ps_list[sg][:], m_t[:, j:j + 2, :],
                            dst[:, j:j + 2, :],
                            start=is_sg_first,
                            stop=is_sg_last,
                            perf_mode=mybir.MatmulPerfMode.DoubleRow,
                            skip_group_check=True,
                        )
                        if warm and j + 2 >= nt:
                            # HAM warm-keeper: one throwaway matmul per
                            # chunk fills the PE idle gap between chunks
                            # so the clock gate stays at 8/8 (idle-gap
                            # oscillation otherwise halves the PE clock
                            # for multi-us stretches)
                            nc.tensor.matmul(
                                ps_w[:], m_t[:, j:j + 2, :],
                                dst[:, j:j + 2, :],
                                start=True, stop=True,
                                perf_mode=mybir.MatmulPerfMode.DoubleRow,
                                skip_group_check=True,
                            )
                        if is_sg_last:
                            # drain this sub-group's psum to SBUF while
                            # the stream continues (the DMA out happens
                            # at the end: a mid-stream dma_start whose
                            # wait is on DVE progress would block the
                            # issuing sequencer and stall every x chunk
                            # queued behind it)
                            nc.vector.tensor_copy(
                                sums_t[SUB * sg:SUB * (sg + 1), :],
                                ps_list[sg][:],
                            )
                if warm:
                    wsink = apool.tile([SUB, 4], f32)
                    nc.vector.tensor_copy(wsink[:], ps_w[:, 0:4])
                # per-sub-group output DMAs, issued after all x chunk
                # DMAs: sg0-2 fire immediately (their copies are long
                # done), only sg3's is on the critical tail
                for sg in range(SPC):
                    nc.sync.dma_start(
                        sums_out[SUB * sg:SUB * (sg + 1), :],
                        sums_t[SUB * sg:SUB * (sg + 1), :],
                    )

    nc.compile()
    return nc


def _pack_positions(scnt):
    """Assign sub-groups to (core, position) by descending-count
    octiles: position p gets the p-th octile of sorted counts, so
    seg[p] = 2*ceil(max/256) over that octile is tight. Returns
    (segs, assign) with assign[core][pos] = sub-group id."""
    order = np.argsort(-scnt)  # descending by count
    segs = []
    assign = [[None] * SPC for _ in range(N_CORES)]
    for p in range(SPC):
        octile = order[p * N_CORES:(p + 1) * N_CORES]
        mx = int(scnt[octile].max())
        segs.append(max(2, 2 * int(np.ceil(mx / 256.0))))
        for c in range(N_CORES):
            assign[c][p] = int(octile[c])
    return tuple(segs), assign


def _route(x, l, segs, assign, ch):
    """Host-side routing: per core, rows of its 4 assigned sub-groups
    in chunk-contiguous partition-major SBUF layout, plus sub-group-
    relative labels."""
    l = np.asarray(l).astype(np.int64).ravel()
    x = np.asarray(x)
    valid = (l >= 0) & (l < C)
    if not valid.all():
        x = x[valid]
        l = l[valid]
    sub = l >> 5
    order = np.argsort(sub, kind="stable")
    scnt = np.bincount(sub, minlength=SPC * N_CORES)
    starts = np.concatenate([[0], np.cumsum(scnt)])

    T = sum(segs)
    bounds = np.cumsum([0] + list(segs))
    entries = _plan_entries(segs, ch)
    n_entries = len(entries)

    xq = x.astype(ml_dtypes.float8_e4m3fn)
    head = min(32, T)
    iota_np = np.ascontiguousarray(
        np.tile(np.arange(SUB, dtype=np.uint8), (128, ch)))

    in_maps = []
    for c in range(N_CORES):
        arr = np.zeros((T * 128, D), dtype=xq.dtype)
        lab_c = np.full(T * 128, 255, dtype=np.int64)
        for p in range(SPC):
            s = assign[c][p]
            rows = order[starts[s]:starts[s + 1]]
            n = len(rows)
            off = bounds[p] * 128
            arr[off:off + n] = xq[rows]
            lab_c[off:off + n] = l[rows] - SUB * s
        arr = arr.reshape(T, 128, D)
        xs_c = np.zeros((n_entries * 128, ch * D), dtype=xq.dtype)
        for k, (t0, nt) in enumerate(entries):
            blk = arr[t0:t0 + nt].transpose(1, 0, 2).reshape(128, nt * D)
            xs_c[k * 128:(k + 1) * 128, 0:nt * D] = blk
        lab_c = np.ascontiguousarray(
            lab_c.reshape(T, 128).T).astype(np.uint8)
        im = {"xs": xs_c,
              "lab_h": np.ascontiguousarray(lab_c[:, 0:head]),
              "iota": iota_np}
        if T > head:
            im["lab_r"] = np.ascontiguousarray(lab_c[:, head:T])
        in_maps.append(im)
    return in_maps


def _epilogue(sums, l, center_img, center_skt):
    ll = np.asarray(l).astype(np.int64).ravel()
    ll = ll[(ll >= 0) & (ll < C)]
    counts = np.bincount(ll, minlength=C)[:C].astype(np.float32)
    cimg = np.asarray(center_img, dtype=np.float32)
    cskt = np.asarray(center_skt, dtype=np.float32)
    present = counts > 0
    mean = sums[:C] / np.maximum(counts, 1.0)[:, None]
    upd = cimg * MOMENTUM + mean * (1.0 - MOMENTUM)
    upd = upd / np.linalg.norm(upd, axis=1, keepdims=True)
    new_img = np.where(present[:, None], upd, cimg)
    diff = new_img - cskt
    sq = np.sum(diff * diff, axis=1)
    n_present = max(float(present.sum()), 1.0)
    return np.float32(np.where(present, sq, 0.0).sum() / n_present)


def _run(x, l, center_img, center_skt, cfg=None, trace=False):
    cfg = dict(cfg or {})
    ch = cfg.setdefault("ch", CH)

    ll = np.asarray(l).astype(np.int64).ravel()
    ll = ll[(ll >= 0) & (ll < C)]
    scnt = np.bincount(ll >> 5, minlength=SPC * N_CORES)
    segs, assign = _pack_positions(scnt)

    in_maps = _route(x, l, segs, assign, ch)

    key = (segs, ch, cfg.get("n_dma", 2), cfg.get("dbufs", 18),
           cfg.get("mbufs", 12), cfg.get("gp_from", GP_FROM),
           cfg.get("n_fresh", N_FRESH), cfg.get("warm", 0),
           cfg.get("primer", 0))
    if key not in _CACHED:
        _CACHED[key] = _build_nc(segs, cfg)
    nc = _CACHED[key]

    res = run_bass_kernel_spmd(
        nc, in_maps, core_ids=list(range(N_CORES)), trace=trace
    )
    # un-permute: core c psum block p holds sub-group assign[c][p]
    sums = np.zeros((SPC * N_CORES * SUB, D), np.float32)
    for c in range(N_CORES):
        rc = res.results[c]["sums"].astype(np.float32)
        for p in range(SPC):
            s = assign[c][p]
            sums[SUB * s:SUB * (s + 1)] = rc[SUB * p:SUB * (p + 1)]
    loss = _epilogue(sums, l, center_img, center_skt)
    return loss, res


def kernel(x, l, center_img, center_skt):
    loss, _ = _run(x, l, center_img, center_skt)
    return np.asarray(loss, dtype=np.float32).reshape(())
